# revision 1
# baseline (speedup 1.0000x reference)
"""LSTMCell (B=16384, IN=HID=512) on 8 TRN2 NeuronCores.

Strategy: data-parallel over batch (2048 rows/core), weights replicated.
Host pre-packs operands so the device kernel needs zero transposes:
  - GEMM computed as gates.T = W_cat.T @ [x;h].T  (K=1024 on partitions)
  - x/h/W cast to bf16 on host (fp32 PSUM accumulation on PE)
  - c / outputs stay fp32
Device per core: 512 matmuls [128k x 128m] @ [128k x 512n] -> PSUM,
ACT evicts PSUM with fused per-partition bias + sigmoid/tanh, DVE does the
elementwise cell update, contiguous DMAs throughout.
"""

import sys

sys.path.insert(0, "/opt/trn_rl_repo")

from contextlib import ExitStack

import ml_dtypes
import numpy as np

import concourse.bass as bass  # noqa: F401  (bass types used via bacc/mybir)
import concourse.mybir as mybir
import concourse.tile as tile
from concourse import bacc
from concourse.bass_utils import run_bass_kernel_spmd

B_FULL, IN, HID = 16384, 512, 512
NCORES = 8
BL = B_FULL // NCORES  # 2048 batch rows per core
JW = 512               # batch columns per chunk (matmul free dim)
P = 128

BF16 = mybir.dt.bfloat16
F32 = mybir.dt.float32
AF = mybir.ActivationFunctionType
BF16_NP = ml_dtypes.bfloat16

NK = (IN + HID) // P   # 8  k-chunks of the contraction dim
NR = HID // P          # 4  row-blocks of H per gate
NM = 4 * HID // P      # 16 gate-row blocks total (i,g,f,o order)


def build_nc(bl=BL):
    """Build the single-core Bass program (SPMD-replicated across cores)."""
    nbn = bl // JW
    nc = bacc.Bacc("TRN2", target_bir_lowering=False, debug=False)

    xh_in = nc.dram_tensor("xh_in", [nbn, NK, P, JW], BF16, kind="ExternalInput")
    wt_in = nc.dram_tensor("wt_in", [NK, P, 4 * HID], BF16, kind="ExternalInput")
    bias_in = nc.dram_tensor("bias_in", [P, NM], F32, kind="ExternalInput")
    c_in = nc.dram_tensor("c_in", [nbn, NR, P, JW], F32, kind="ExternalInput")
    h_out = nc.dram_tensor("h_out", [nbn, NR, P, JW], F32, kind="ExternalOutput")
    c_out = nc.dram_tensor("c_out", [nbn, NR, P, JW], F32, kind="ExternalOutput")

    with ExitStack() as ctx:
        tc = ctx.enter_context(tile.TileContext(nc))
        wpool = ctx.enter_context(tc.tile_pool(name="w", bufs=1))
        xpool = ctx.enter_context(tc.tile_pool(name="xh", bufs=2))
        cpool = ctx.enter_context(tc.tile_pool(name="cin", bufs=3))
        gpool = ctx.enter_context(tc.tile_pool(name="gates", bufs=3))
        opool = ctx.enter_context(tc.tile_pool(name="outs", bufs=3))
        pspool = ctx.enter_context(tc.tile_pool(name="ps", bufs=2, space="PSUM"))

        # PE HAM warmup: the first ~11.7us of the kernel is DMA/runtime setup
        # with the PE idle, which leaves the PE clock gated at 1.2 GHz when
        # real matmuls start. Dummy matmuls (no DMA deps) keep the activity
        # monitor busy through that window so real work runs at 2.4 GHz from
        # the first tile. 20 MMs ~= 6us busy: enough to latch the HAM warm
        # (8 MMs / 3.4us measured insufficient) without delaying real work
        # much past the ~11.7us data-ready point (36 MMs overflowed to 18us).
        wu = wpool.tile([P, JW], BF16, tag="wu", name="wu")
        nc.vector.memset(wu[:], 0.0)
        wu_ps = pspool.tile([P, JW], F32, tag="ps0", name="wu_ps")
        for _ in range(20):
            nc.tensor.matmul(wu_ps[:], wu[:, :P], wu[:], start=True, stop=True)

        # Weights + bias resident in SBUF for the whole kernel. Loaded on the
        # gpsimd DMA queue, in parallel with the sync-queue xh/c streams, so
        # the first matmul isn't serialized behind the full 4MB weight load.
        wts = [None] * NK
        bias_t = None

        for nb in range(nbn):
            xh_tiles = []
            for k in range(NK):
                xt = xpool.tile([P, JW], BF16, tag=f"xh{k}", name=f"xh{k}")
                nc.sync.dma_start(xt[:], xh_in[nb, k])
                xh_tiles.append(xt)
            if nb == 0:
                for k in range(NK):
                    wt = wpool.tile([P, 4 * HID], BF16, tag=f"w{k}", name=f"w{k}")
                    nc.gpsimd.dma_start(wt[:], wt_in[k])
                    wts[k] = wt
                bias_t = wpool.tile([P, NM], F32, tag="bias", name="bias")
                nc.gpsimd.dma_start(bias_t[:], bias_in[:])
            for r in range(NR):
                ct = cpool.tile([P, JW], F32, tag="c")
                nc.sync.dma_start(ct[:], c_in[nb, r])
                ps = [
                    pspool.tile([P, JW], F32, tag=f"ps{g}", name=f"ps{g}")
                    for g in range(4)
                ]
                for g in range(4):
                    m = g * NR + r
                    for k in range(NK):
                        nc.tensor.matmul(
                            ps[g][:],
                            wts[k][:, m * P : (m + 1) * P],
                            xh_tiles[k][:],
                            start=(k == 0),
                            stop=(k == NK - 1),
                        )
                it = gpool.tile([P, JW], F32, tag="i")
                gt = gpool.tile([P, JW], F32, tag="g")
                ft = gpool.tile([P, JW], F32, tag="f")
                ot = gpool.tile([P, JW], F32, tag="o")
                nc.scalar.activation(
                    it[:], ps[0][:], AF.Sigmoid, bias=bias_t[:, 0 + r : 1 + r]
                )
                nc.scalar.activation(
                    gt[:], ps[1][:], AF.Tanh, bias=bias_t[:, NR + r : NR + r + 1]
                )
                nc.scalar.activation(
                    ft[:], ps[2][:], AF.Sigmoid,
                    bias=bias_t[:, 2 * NR + r : 2 * NR + r + 1],
                )
                nc.scalar.activation(
                    ot[:], ps[3][:], AF.Sigmoid,
                    bias=bias_t[:, 3 * NR + r : 3 * NR + r + 1],
                )
                t1 = gpool.tile([P, JW], F32, tag="t1")
                t2 = gpool.tile([P, JW], F32, tag="t2")
                cn = opool.tile([P, JW], F32, tag="cn")
                tch = gpool.tile([P, JW], F32, tag="tch")
                hn = opool.tile([P, JW], F32, tag="hn")
                nc.vector.tensor_mul(t1[:], it[:], gt[:])
                nc.vector.tensor_mul(t2[:], ft[:], ct[:])
                nc.vector.tensor_add(cn[:], t1[:], t2[:])
                nc.scalar.activation(tch[:], cn[:], AF.Tanh)
                nc.vector.tensor_mul(hn[:], ot[:], tch[:])
                nc.sync.dma_start(c_out[nb, r], cn[:])
                nc.sync.dma_start(h_out[nb, r], hn[:])
    nc.compile()
    return nc


def prep_shared(Wxi, Wxg, Wxf, Wxo, Whi, Whg, Whf, Who, bias_sum):
    """wt_in [NK,P,4H] bf16 and bias_in [P,NM] f32 (gate order i,g,f,o)."""
    Wx = np.concatenate([Wxi, Wxg, Wxf, Wxo], axis=0)  # [4H, IN]
    Wh = np.concatenate([Whi, Whg, Whf, Who], axis=0)  # [4H, HID]
    WT = np.concatenate([Wx.T, Wh.T], axis=0)          # [K=1024, 4H]
    wt_arr = np.ascontiguousarray(
        WT.reshape(NK, P, 4 * HID).astype(BF16_NP)
    )
    bias_arr = np.ascontiguousarray(
        bias_sum.reshape(NM, P).T.astype(np.float32)
    )
    return wt_arr, bias_arr


def prep_core(x_s, h_s, c_s):
    """Per-core xh_in [nb,NK,P,JW] bf16 and c_in [nb,NR,P,JW] f32."""
    bl = x_s.shape[0]
    nbn = bl // JW
    xhT = np.concatenate([x_s, h_s], axis=1).T  # [K=1024, bl]
    xh_arr = np.ascontiguousarray(
        xhT.reshape(NK, P, nbn, JW).transpose(2, 0, 1, 3).astype(BF16_NP)
    )
    cT = c_s.T  # [HID, bl]
    c_arr = np.ascontiguousarray(
        cT.reshape(NR, P, nbn, JW).transpose(2, 0, 1, 3).astype(np.float32)
    )
    return xh_arr, c_arr


def post_core(arr):
    """[nb,NR,P,JW] -> [bl, HID]"""
    arr = np.asarray(arr)
    nbn = arr.size // (NR * P * JW)
    arr = arr.reshape(nbn, NR, P, JW)
    return arr.transpose(0, 3, 1, 2).reshape(nbn * JW, HID)


_NC_CACHE = {}


def _get_nc(bl=BL):
    if bl not in _NC_CACHE:
        _NC_CACHE[bl] = build_nc(bl)
    return _NC_CACHE[bl]


def make_in_maps(x, h, c, Wxi, bxi, Wxo, bxo, Wxf, bxf, Wxg, bxg,
                 Whi, bhi, Who, bho, Whf, bhf, Whg, bhg, ncores=NCORES):
    bias_sum = np.concatenate(
        [bxi + bhi, bxg + bhg, bxf + bhf, bxo + bho], axis=0
    ).astype(np.float32)
    wt_arr, bias_arr = prep_shared(Wxi, Wxg, Wxf, Wxo, Whi, Whg, Whf, Who, bias_sum)
    bl = x.shape[0] // ncores
    in_maps = []
    for i in range(ncores):
        s = slice(i * bl, (i + 1) * bl)
        xh_arr, c_arr = prep_core(
            np.asarray(x[s], np.float32),
            np.asarray(h[s], np.float32),
            np.asarray(c[s], np.float32),
        )
        in_maps.append(
            {"xh_in": xh_arr, "wt_in": wt_arr, "bias_in": bias_arr, "c_in": c_arr}
        )
    return in_maps


def kernel(x, h, c, Wxi, bxi, Wxo, bxo, Wxf, bxf, Wxg, bxg,
           Whi, bhi, Who, bho, Whf, bhf, Whg, bhg):
    args = dict(
        x=np.asarray(x, np.float32), h=np.asarray(h, np.float32),
        c=np.asarray(c, np.float32),
        Wxi=np.asarray(Wxi, np.float32), bxi=np.asarray(bxi, np.float32),
        Wxo=np.asarray(Wxo, np.float32), bxo=np.asarray(bxo, np.float32),
        Wxf=np.asarray(Wxf, np.float32), bxf=np.asarray(bxf, np.float32),
        Wxg=np.asarray(Wxg, np.float32), bxg=np.asarray(bxg, np.float32),
        Whi=np.asarray(Whi, np.float32), bhi=np.asarray(bhi, np.float32),
        Who=np.asarray(Who, np.float32), bho=np.asarray(bho, np.float32),
        Whf=np.asarray(Whf, np.float32), bhf=np.asarray(bhf, np.float32),
        Whg=np.asarray(Whg, np.float32), bhg=np.asarray(bhg, np.float32),
    )
    in_maps = make_in_maps(**args)
    nc = _get_nc(BL)
    res = run_bass_kernel_spmd(nc, in_maps, core_ids=list(range(NCORES)))
    h_new = np.empty((B_FULL, HID), np.float32)
    c_new = np.empty((B_FULL, HID), np.float32)
    for i in range(NCORES):
        s = slice(i * BL, (i + 1) * BL)
        h_new[s] = post_core(res.results[i]["h_out"])
        c_new[s] = post_core(res.results[i]["c_out"])
    return (h_new, c_new)

